# revision 13
# baseline (speedup 1.0000x reference)
"""Trainium2 Bass kernel for nn_Net_21852793602541 (gnn_message_passing).

The reference net's output depends only on a tiny dependency cone of the
message-passing graph: the final hidden layer reads the wave-2 snapshot of
neuron activations, so only neurons feeding neuron 255 through channels whose
source was already processed matter.  For the fixed graph that is a 3-conv
chain (x -> n0 -> n172 -> n215), one 784->200 FC block, a 200->10 FC and
log_softmax.  The cone is recomputed at runtime from the src/tgt inputs.

Per-core mapping (data-parallel over batch, 16 images/core on 8 cores):
  * 5x5 conv == one PE accumulation group: contraction K = (dy, slot-row)
    with a banded-Toeplitz stationary (fp16) against 5 y-shifted slot copies
    of the padded image block (fp16), N = (batch, y) = 448.  The B-part
    stationary carries a 33rd row holding the conv bias against an all-ones
    row of the moving operand, so PSUM already contains the bias and the
    relu is a plain max(0, x).
  * the relu is split PSUM-side between ACT and DVE (y halves); the slot
    replication copies are spread over DVE (3) and GPSIMD (1); the B-part
    matmul of the next conv is issued first so it can start on the earliest
    copy.
  * fc1 streams the 200 hidden units as the moving operand (7 accumulated
    matmuls); fc1 bias rides a ones-row of the fc stack.  The [16, 200]
    result is relu'd on DVE, transposed via PE, and fc2 uses the hidden
    vectors as the *stationary* so logits land [batch, cls] with no final
    transpose; fc2 bias rides a ones-row of the second hidden chunk.
  * log_softmax without max-subtraction (logits are O(5)): ACT exp, DVE
    reduce, ACT ln, one fused DVE subtract.
  * a Bacc subclass pins the activation-table pass to the
    natural_log_exp_and_others set (covers Relu/Identity/Exp/Ln/Copy), so
    exactly one ACT_TABLE_LOAD runs, early and off the critical path.
"""

import numpy as np

import concourse.bass as bass
import concourse.tile as tile
from concourse import bacc, mybir
from concourse.bacc import _bass_rust
from concourse.bass_utils import run_bass_kernel_spmd
from concourse.hw_specs import get_activation_tables

# The axon NTFF profile hook normally lives in antenv.axon_hooks, which this
# image lacks.  Shim it from the boot module's ctypes implementation so
# BASS_TRACE=1 profiling works; degrade silently if unavailable.
try:
    import antenv.axon_hooks  # noqa: F401
except ImportError:
    try:
        import sys as _sys
        import types as _types

        from trn_agent_boot.trn_boot import _ntff_profile_via_ctypes

        _hook = _ntff_profile_via_ctypes('/opt/axon/libaxon_pjrt.so')
        _mod = _types.ModuleType('antenv.axon_hooks')
        _mod.get_axon_ntff_profile_hook = lambda: _hook
        _mod.set_axon_ntff_profile_hook = lambda h: None
        _sys.modules['antenv.axon_hooks'] = _mod
    except Exception:
        pass

F32 = mybir.dt.float32
F16 = mybir.dt.float16
AF = mybir.ActivationFunctionType
ALU = mybir.AluOpType
N_NEURONS = 256
N_CORES = 8
B_TOTAL = 128
B = B_TOTAL // N_CORES  # 16 images per core
HW = 28
FC_HID = 200
N_CLS = 10

_ACT_SET = 'natural_log_exp_and_others'

LAST_RESULT = None  # BassKernelResults of the most recent run (for profiling)


class _Bacc(bacc.Bacc):
    """Bacc whose activation-table pass only ever picks the combined
    exp+ln set, so the whole kernel needs a single ACT_TABLE_LOAD (the
    greedy first-match pass would otherwise load exp_and_others early and
    natural_log right between the tail's Exp and Ln)."""

    def insert_act_table_loads(self):
        has_activation = any(
            isinstance(i, mybir.InstActivation)
            for b in self.main_func.blocks
            for i in b.instructions
        )
        if not has_activation:
            return
        tables = get_activation_tables(self.m.arch)
        filt = [(name, (fns if name == _ACT_SET else set()))
                for name, fns in tables.items()]
        _bass_rust.insert_act_table_loads(self, filt)


# ---------------------------------------------------------------- schedule
def _schedule(src, tgt):
    n = N_NEURONS
    in_lists = [src[np.where(tgt == i)[0]].astype(np.int64).tolist() for i in range(n)]
    waves = []
    processed = np.zeros(n, bool)
    frontier = [0]
    while True:
        waves.append(list(frontier))
        processed[frontier] = True
        if processed[n - 1]:
            break
        nxt = set()
        for v in frontier:
            for m in tgt[src == v]:
                if not processed[m]:
                    nxt.add(int(m))
        frontier = sorted(nxt)
        assert frontier, "last neuron unreachable"
    return in_lists, waves


def _cone(src, tgt):
    """Returns (steps, fc_live).

    steps: ordered list of (node, [(srckey, channel), ...]) where srckey is
      'x' for the image input or an int neuron id computed in an earlier step.
    fc_live: [(channel_of_255, src_node), ...] live channels of the readout.
    """
    n = N_NEURONS
    in_lists, waves = _schedule(src, tgt)
    wave_of = {}
    for wi, w in enumerate(waves):
        for v in w:
            if v not in wave_of:
                wave_of[v] = wi
    BIG = 1 << 30
    w255 = wave_of[n - 1]
    fc_live = [(c, int(s)) for c, s in enumerate(in_lists[n - 1])
               if wave_of.get(int(s), BIG) < w255]

    live = {}
    stack = [s for _, s in fc_live]
    seen = set()
    while stack:
        v = stack.pop()
        if v in seen:
            continue
        seen.add(v)
        if v == 0:
            live[0] = [('x', 0)]
            continue
        chans = [(int(s), c) for c, s in enumerate(in_lists[v])
                 if wave_of.get(int(s), BIG) < wave_of[v]]
        assert chans, f"cone node {v} has no live channels"
        live[v] = [(s, c) for s, c in chans]
        stack += [s for s, _ in chans]

    steps = sorted(live.items(), key=lambda kv: wave_of[kv[0]])
    return steps, fc_live


# ---------------------------------------------------------- host-side packing
def _toeplitz(w):
    """w [5,5] -> [160, 28] banded matrix over K=(dy, row).

    Slot row r of each 32-row group holds padded-image column (r+2) mod 32,
    so the activation value at x lands at row x (32-aligned writes; wrapped
    rows 28..31 hold the zero x-padding)."""
    T = np.zeros((160, HW), np.float32)
    for dy in range(5):
        for dx in range(5):
            for xc in range(HW):
                T[dy * 32 + (xc + dx - 2) % 32, xc] = w[dy, dx]
    return T


def _xstack(xb):
    """xb [B,28,28] -> [160, B*32] fp16: five y-shifted padded slot copies.

    Slot_dy[r, b*32+yp] = xpad[b, yp+dy-2, (r+2) % 32]."""
    xpad = np.zeros((B, 32, 32), np.float32)
    xpad[:, 2:30, 2:30] = xb
    st = np.zeros((5, 32, B, 32), np.float32)
    for dy in range(5):
        lo, hi = max(0, 2 - dy), min(32, 34 - dy)
        st[dy, :, :, lo:hi] = xpad[:, lo + dy - 2:hi + dy - 2, :].transpose(2, 0, 1)
    st = np.roll(st, -2, axis=1)
    return st.reshape(160, B * 32).astype(np.float16)


def _pack(steps, fc_live, conv_w, conv_b, fc1_w, fc1_b, fc2_w, fc2_b):
    """Builds mainh (f16, [128, *]), tb (f16, [33, *]), consts (f32,
    [128, 36]), f1w (f16, [128, 1400*nfc]) and the slot map."""
    slots = {}
    col = 0
    for v, chans in steps:
        for j, _ in enumerate(chans):
            slots[('toep', v, j)] = col
            col += HW
    a_cols = col
    slots['xs'] = a_cols
    n_toep = a_cols  # same column budget in tb (B parts align with A parts)

    mainh = np.zeros((128, a_cols + 512), np.float16)
    tb = np.zeros((32, a_cols + 512), np.float16)
    for v, chans in steps:
        for j, (skey, ch) in enumerate(chans):
            T = _toeplitz(conv_w[v, 0, ch])
            c0 = slots[('toep', v, j)]
            mainh[:, c0:c0 + HW] = T[:128]
            tb[:, c0:c0 + HW] = T[128:]

    # consts: identity (16x16), fc2wA [128,10], fc2wB [96,10] w/ bias row;
    # conv biases ride a separate tiny tensor so they land early
    nsteps = len(steps)
    consts = np.zeros((128, 36), np.float32)
    consts[:B, 0:B] = np.eye(B, dtype=np.float32)
    w2t = fc2_w.T  # [200, 10]
    consts[:, 16:26] = w2t[:128]
    consts[:FC_HID - 128, 26:36] = w2t[128:]
    consts[72, 26:36] = fc2_b
    cbt = np.zeros((HW, nsteps), np.float32)
    for i, (v, _) in enumerate(steps):
        slots[('cb', v)] = i
        cbt[:, i] = conv_b[v]

    n_fc = len(fc_live)
    f1p = np.zeros((128, 1400 * n_fc), np.float16)
    for k, (c, s) in enumerate(fc_live):
        blk = fc1_w[:, c * 784:(c + 1) * 784].reshape(FC_HID, HW, HW)  # [h, y, x]
        arr = blk.reshape(FC_HID, 4, 7, HW).transpose(1, 3, 2, 0)  # [yg, x, ysub, h]
        f1p[:, k * 1400:(k + 1) * 1400] = np.pad(
            arr, ((0, 0), (0, 4), (0, 0), (0, 0))).reshape(128, 1400)
    f1p[28, 0:FC_HID] = fc1_b  # vs ones-row of fc stack (k=0, sj=0 only)
    return mainh, tb, consts, cbt, f1p, slots


# ---------------------------------------------------------- device program
def _build(steps, fc_live, ncolsA, nfc):
    nc = _Bacc("TRN2", target_bir_lowering=False)
    nsteps = len(steps)
    mainh_d = nc.dram_tensor("mainh", [128, ncolsA + 512], F16, kind="ExternalInput")
    tb_d = nc.dram_tensor("tb", [32, ncolsA + 512], F16, kind="ExternalInput")
    consts_d = nc.dram_tensor("consts", [128, 36], F32, kind="ExternalInput")
    cbt_d = nc.dram_tensor("cbt", [HW, nsteps], F32, kind="ExternalInput")
    f1w_d = nc.dram_tensor("f1w", [128, 1400 * nfc], F16, kind="ExternalInput")
    out_d = nc.dram_tensor("out", [B, N_CLS], F32, kind="ExternalOutput")

    feeds_conv = set()
    for v, chans in steps:
        for skey, _ in chans:
            if skey != 'x':
                feeds_conv.add(skey)
    fc_srcs = [s for _, s in fc_live]
    SL = _SLOTS
    YS = 12  # relu y-split point: ACT gets [0, YS), DVE [YS, 28)

    with tile.TileContext(nc) as tc:
        with (
            tc.tile_pool(name="persist", bufs=1) as pool,
            tc.tile_pool(name="cpsum", bufs=1, space="PSUM") as cpp,
            tc.tile_pool(name="qpsum", bufs=1, space="PSUM") as qpp,
            tc.tile_pool(name="fpsum", bufs=1, space="PSUM") as fpp,
        ):
            mainh = pool.tile([128, ncolsA + 512], F16, tag="mainh")
            tbt = pool.tile([32, ncolsA + 512], F16, tag="tb")
            consts = pool.tile([128, 36], F32, tag="consts")
            cbt = pool.tile([HW, nsteps], F32, tag="cbt")
            f1w = pool.tile([128, 1400 * nfc], F16, tag="f1w")
            # mainh gates conv0; f1w is not needed until fc1 -- both on the
            # HWDGE sync queue (mainh first), small tensors on scalar.
            nc.sync.dma_start(mainh[:], mainh_d[:])
            nc.sync.dma_start(f1w[:], f1w_d[:])
            nc.scalar.dma_start(cbt[:], cbt_d[:])
            nc.scalar.dma_start(tbt[:], tb_d[:])
            nc.scalar.dma_start(consts[:], consts_d[:])

            # dummy activation: hoists the single ACT table load to the top
            swu = pool.tile([1, 2], F32, tag="swu")
            nc.gpsimd.memset(swu[:], 1.0)
            nc.scalar.activation(swu[:, 0:1], swu[:, 0:1], AF.Exp)

            # activation slot tiles per conv producer (fp16, zero borders);
            # memsets split DVE/GPSIMD so neither engine's prologue is long
            stacks = {}
            ms_engines = [nc.vector, nc.gpsimd]
            for mi, v in enumerate(sorted(feeds_conv)):
                eng = ms_engines[mi % 2]
                a = pool.tile([128, B * 32], F16, name=f"stA_{v}", tag=f"stA_{v}")
                b = pool.tile([32, B * 32], F16, name=f"stB_{v}", tag=f"stB_{v}")
                eng.memset(a[:], 0.0)
                eng.memset(b[:], 0.0)
                stacks[v] = (a, b)
            fcstacks = {}
            for sv in sorted(set(fc_srcs)):
                t = pool.tile([128, B * 7], F16, name=f"fcst_{sv}", tag=f"fcst_{sv}")
                nc.gpsimd.memset(t[:], 0.0)
                fv = t[:].rearrange("p (b s) -> p b s", s=7)
                # ones-row for the fc1 bias trick; partition starts must be
                # 32-aligned, so write rows 0:32 -- rows 0:28 are overwritten
                # by the g=0 quarter write, rows 29:31 face zero-padded
                # stationary rows
                nc.gpsimd.memset(fv[0:32, :, 0:1], 1.0)
                fcstacks[sv] = t
            h2 = pool.tile([96, B], F32, tag="h2")
            nc.gpsimd.memset(h2[64:96, :], 1.0)

            xsa = mainh[:, SL['xs']:SL['xs'] + 512]
            xsb = tbt[:, SL['xs']:SL['xs'] + 512]

            def slot_slices(key):
                if key == 'x':
                    av, bv = xsa, xsb
                else:
                    a, b = stacks[key]
                    av, bv = a[:], b[:]
                return (av.rearrange("p (b y) -> p b y", y=32),
                        bv.rearrange("p (b y) -> p b y", y=32))

            # --- conv chain ---
            for v, chans in steps:
                nch = len(chans)
                cb = cbt[:, SL[('cb', v)]:SL[('cb', v)] + 1]
                fc_only = v in fcstacks and v not in feeds_conv

                if fc_only:
                    # four independent quarter PSUMs: each regroup write has
                    # its own tile, so the DVE/ACT pairs truly overlap
                    fst = fcstacks[v]
                    fv = fst[:].rearrange("p (b s) -> p b s", s=7)
                    # four PSUM tiles in four banks: concurrently-open
                    # accumulation groups must not share a PSUM bank, and
                    # separate tiles keep the regroup writes parallel
                    qs = [qpp.tile([HW, B * 7], F32, tag=f"q{g}",
                                   name=f"q{v}_{g}")[:]
                          for g in range(4)]
                    for j, (skey, ch) in enumerate(chans):
                        c0 = SL[('toep', v, j)]
                        av, bv = slot_slices(skey)
                        for g in range(4):
                            ysl = slice(2 + 7 * g, 9 + 7 * g)
                            nc.tensor.matmul(qs[g], tbt[:, c0:c0 + HW],
                                             bv[:, :, ysl],
                                             start=(j == 0), stop=False)
                        for g in range(4):
                            ysl = slice(2 + 7 * g, 9 + 7 * g)
                            nc.tensor.matmul(qs[g], mainh[:, c0:c0 + HW],
                                             av[:, :, ysl],
                                             start=False, stop=(j == nch - 1))
                    for g in range(4):
                        dst = fv[g * 32:g * 32 + HW, :, :]
                        qv = qs[g].rearrange("p (b y) -> p b y", y=7)
                        if g % 2 == 0:
                            nc.vector.tensor_scalar(dst, qv, cb, 0.0,
                                                    ALU.add, ALU.max)
                        else:
                            nc.scalar.activation(dst, qv, AF.Relu, bias=cb,
                                                 scale=1.0)
                    continue

                # y-split PSUM halves: ACT relus the low half while DVE does
                # the high half -- separate tiles, so no reader serialization
                plo = cpp.tile([HW, B * YS], F32, tag="pslo", name=f"plo{v}")
                phi = cpp.tile([HW, B * (HW - YS)], F32, tag="pshi",
                               name=f"phi{v}")
                for j, (skey, ch) in enumerate(chans):
                    c0 = SL[('toep', v, j)]
                    av, bv = slot_slices(skey)
                    # B part first: its moving operand is copied first, so
                    # the PE can start before the dy slot copies finish
                    nc.tensor.matmul(plo[:], tbt[:, c0:c0 + HW],
                                     bv[:, :, 2:2 + YS],
                                     start=(j == 0), stop=False)
                    nc.tensor.matmul(phi[:], tbt[:, c0:c0 + HW],
                                     bv[:, :, 2 + YS:30],
                                     start=(j == 0), stop=False)
                    nc.tensor.matmul(plo[:], mainh[:, c0:c0 + HW],
                                     av[:, :, 2:2 + YS],
                                     start=False, stop=(j == nch - 1))
                    nc.tensor.matmul(phi[:], mainh[:, c0:c0 + HW],
                                     av[:, :, 2 + YS:30],
                                     start=False, stop=(j == nch - 1))
                plov = plo[:].rearrange("p (b y) -> p b y", y=YS)
                phiv = phi[:].rearrange("p (b y) -> p b y", y=HW - YS)

                av, bv = slot_slices(v)
                g2 = av[64:64 + HW, :, 2:30]
                nc.scalar.activation(av[64:64 + HW, :, 2:2 + YS], plov,
                                     AF.Relu, bias=cb, scale=1.0)
                nc.vector.tensor_scalar(av[64:64 + HW, :, 2 + YS:30], phiv,
                                        cb, 0.0, ALU.add, ALU.max)
                nc.vector.tensor_copy(bv[0:HW, :, 0:28], g2)
                nc.vector.tensor_copy(av[0:HW, :, 4:32], g2)
                nc.vector.tensor_copy(av[32:32 + HW, :, 3:31], g2)
                nc.scalar.copy(av[96:96 + HW, :, 1:29], g2)
                if v in fcstacks:  # node feeds both conv and fc (rare)
                    fst = fcstacks[v]
                    fv = fst[:].rearrange("p (b s) -> p b s", s=7)
                    for g in range(4):
                        dst = fv[g * 32:g * 32 + HW, :, :]
                        lo, hi = 7 * g, 7 * g + 7
                        if hi <= YS:
                            src3 = plov[:, :, lo:hi]
                        elif lo >= YS:
                            src3 = phiv[:, :, lo - YS:hi - YS]
                        else:
                            nc.vector.tensor_scalar(dst[:, :, 0:YS - lo],
                                                    plov[:, :, lo:YS], cb,
                                                    0.0, ALU.add, ALU.max)
                            nc.vector.tensor_scalar(dst[:, :, YS - lo:],
                                                    phiv[:, :, 0:hi - YS], cb,
                                                    0.0, ALU.add, ALU.max)
                            continue
                        nc.vector.tensor_scalar(dst, src3, cb, 0.0,
                                                ALU.add, ALU.max)

            # --- fc1: activations stationary, hidden units streamed ---
            p1 = fpp.tile([B, FC_HID], F32, tag="p1")
            for k in range(nfc):
                fst = fcstacks[fc_live[k][1]]
                fv = fst[:].rearrange("p (b s) -> p b s", s=7)
                for sj in range(7):
                    i = k * 7 + sj
                    nc.tensor.matmul(p1[:], fv[:, :, sj:sj + 1],
                                     f1w[:, (k * 7 + sj) * 200:(k * 7 + sj + 1) * 200],
                                     start=(i == 0), stop=(i == 7 * nfc - 1))
            # bias already accumulated; relu+copy in one DVE op
            ht = pool.tile([B, FC_HID], F32, tag="ht")
            nc.vector.tensor_scalar_max(ht[:], p1[:], 0.0)
            idn = consts[:B, 0:B]
            # t1/t2/ps2 share one PSUM bank: their matmul groups never
            # overlap in time (transposes close before fc2 starts)
            t12 = fpp.tile([128, 2 * B + N_CLS], F32, tag="t12")
            nc.tensor.transpose(t12[:, 0:B], ht[:, 0:128], idn)
            nc.tensor.transpose(t12[0:FC_HID - 128, B:2 * B],
                                ht[:, 128:FC_HID], idn)
            h1 = pool.tile([128, B], F32, tag="h1")
            nc.vector.tensor_copy(h1[:], t12[:, 0:B])
            nc.scalar.copy(h2[0:FC_HID - 128, :],
                           t12[0:FC_HID - 128, B:2 * B])

            # --- fc2 (hidden stationary -> logits [b, cls]) + log_softmax ---
            ps2 = t12[0:B, 2 * B:2 * B + N_CLS]
            nc.tensor.matmul(ps2, h1[:], consts[:, 16:26],
                             start=True, stop=False)
            nc.tensor.matmul(ps2, h2[:], consts[0:96, 26:36],
                             start=False, stop=True)
            ex = pool.tile([B, N_CLS], F32, tag="ex")
            nc.scalar.activation(ex[:], ps2, AF.Exp)
            sm = pool.tile([B, 1], F32, tag="sm")
            nc.vector.reduce_sum(sm[:], ex[:], axis=mybir.AxisListType.X)
            lse = pool.tile([B, 1], F32, tag="lse")
            nc.scalar.activation(lse[:], sm[:], AF.Ln)
            res = pool.tile([B, N_CLS], F32, tag="res")
            nc.vector.tensor_scalar_sub(res[:], ps2, lse[:])
            nc.sync.dma_start(out_d[:], res[:])
    nc.compile()
    return nc


_SLOTS = None
_PROG_CACHE = {}


def kernel(x, src, tgt, conv_w, conv_b, fc1_w, fc1_b, fc2_w, fc2_b):
    global _SLOTS, LAST_RESULT
    x = np.asarray(x, np.float32)
    src = np.asarray(src, np.int32)
    tgt = np.asarray(tgt, np.int32)
    conv_w = np.asarray(conv_w, np.float32)
    conv_b = np.asarray(conv_b, np.float32)
    fc1_w = np.asarray(fc1_w, np.float32)
    fc1_b = np.asarray(fc1_b, np.float32)
    fc2_w = np.asarray(fc2_w, np.float32)
    fc2_b = np.asarray(fc2_b, np.float32)

    steps, fc_live = _cone(src, tgt)
    mainh0, tb, consts, cbt, f1p, slots = _pack(steps, fc_live, conv_w, conv_b,
                                                fc1_w, fc1_b, fc2_w, fc2_b)
    _SLOTS = slots
    ncolsA = slots['xs']

    key = (tuple((v, tuple(ch)) for v, ch in steps), tuple(fc_live), ncolsA)
    if key not in _PROG_CACHE:
        _PROG_CACHE[key] = _build(steps, fc_live, ncolsA, len(fc_live))
    nc = _PROG_CACHE[key]

    xs = x[:, 0]  # [128, 28, 28]
    in_maps = []
    for c in range(N_CORES):
        st = _xstack(xs[c * B:(c + 1) * B])
        mainh = mainh0.copy()
        mainh[:, ncolsA:ncolsA + 512] = st[:128]
        tbc = tb.copy()
        tbc[:, ncolsA:ncolsA + 512] = st[128:160]
        in_maps.append({"mainh": mainh, "tb": tbc, "consts": consts,
                        "cbt": cbt, "f1w": f1p})

    LAST_RESULT = run_bass_kernel_spmd(nc, in_maps, list(range(N_CORES)))
    out = np.concatenate([r["out"] for r in LAST_RESULT.results], axis=0)
    return out.astype(np.float32)


# revision 14
# speedup vs baseline: 1.1395x; 1.1395x over previous
"""Trainium2 Bass kernel for nn_Net_21852793602541 (gnn_message_passing).

The reference net's output depends only on a tiny dependency cone of the
message-passing graph: the final hidden layer reads the wave-2 snapshot of
neuron activations, so only neurons feeding neuron 255 through channels whose
source was already processed matter.  For the fixed graph that is a 3-conv
chain (x -> n0 -> n172 -> n215), one 784->200 FC block, a 200->10 FC and
log_softmax.  The cone is recomputed at runtime from the src/tgt inputs.

Per-core mapping (data-parallel over batch, 16 images/core on 8 cores):
  * 5x5 conv == one PE accumulation group: contraction K = (dy, slot-row)
    with a banded-Toeplitz stationary (fp16) against 5 y-shifted slot copies
    of the padded image block (fp16), N = (batch, y) = 448.  The B-part
    stationary carries a 33rd row holding the conv bias against an all-ones
    row of the moving operand, so PSUM already contains the bias and the
    relu is a plain max(0, x).
  * the relu is split PSUM-side between ACT and DVE (y halves); the slot
    replication copies are spread over DVE (3) and GPSIMD (1); the B-part
    matmul of the next conv is issued first so it can start on the earliest
    copy.
  * fc1 streams the 200 hidden units as the moving operand (7 accumulated
    matmuls); fc1 bias rides a ones-row of the fc stack.  The [16, 200]
    result is relu'd on DVE, transposed via PE, and fc2 uses the hidden
    vectors as the *stationary* so logits land [batch, cls] with no final
    transpose; fc2 bias rides a ones-row of the second hidden chunk.
  * log_softmax without max-subtraction (logits are O(5)): ACT exp, DVE
    reduce, ACT ln, one fused DVE subtract.
  * a Bacc subclass pins the activation-table pass to the
    natural_log_exp_and_others set (covers Relu/Identity/Exp/Ln/Copy), so
    exactly one ACT_TABLE_LOAD runs, early and off the critical path.
"""

import numpy as np

import concourse.bass as bass
import concourse.tile as tile
from concourse import bacc, mybir
from concourse.bacc import _bass_rust
from concourse.bass_utils import run_bass_kernel_spmd
from concourse.hw_specs import get_activation_tables

# The axon NTFF profile hook normally lives in antenv.axon_hooks, which this
# image lacks.  Shim it from the boot module's ctypes implementation so
# BASS_TRACE=1 profiling works; degrade silently if unavailable.
try:
    import antenv.axon_hooks  # noqa: F401
except ImportError:
    try:
        import sys as _sys
        import types as _types

        from trn_agent_boot.trn_boot import _ntff_profile_via_ctypes

        _hook = _ntff_profile_via_ctypes('/opt/axon/libaxon_pjrt.so')
        _mod = _types.ModuleType('antenv.axon_hooks')
        _mod.get_axon_ntff_profile_hook = lambda: _hook
        _mod.set_axon_ntff_profile_hook = lambda h: None
        _sys.modules['antenv.axon_hooks'] = _mod
    except Exception:
        pass

F32 = mybir.dt.float32
F16 = mybir.dt.float16
AF = mybir.ActivationFunctionType
ALU = mybir.AluOpType
N_NEURONS = 256
N_CORES = 8
B_TOTAL = 128
B = B_TOTAL // N_CORES  # 16 images per core
HW = 28
FC_HID = 200
N_CLS = 10

_ACT_SET = 'natural_log_exp_and_others'

LAST_RESULT = None  # BassKernelResults of the most recent run (for profiling)


class _Bacc(bacc.Bacc):
    """Bacc whose activation-table pass only ever picks the combined
    exp+ln set, so the whole kernel needs a single ACT_TABLE_LOAD (the
    greedy first-match pass would otherwise load exp_and_others early and
    natural_log right between the tail's Exp and Ln)."""

    def insert_act_table_loads(self):
        has_activation = any(
            isinstance(i, mybir.InstActivation)
            for b in self.main_func.blocks
            for i in b.instructions
        )
        if not has_activation:
            return
        tables = get_activation_tables(self.m.arch)
        filt = [(name, (fns if name == _ACT_SET else set()))
                for name, fns in tables.items()]
        _bass_rust.insert_act_table_loads(self, filt)


# ---------------------------------------------------------------- schedule
def _schedule(src, tgt):
    n = N_NEURONS
    in_lists = [src[np.where(tgt == i)[0]].astype(np.int64).tolist() for i in range(n)]
    waves = []
    processed = np.zeros(n, bool)
    frontier = [0]
    while True:
        waves.append(list(frontier))
        processed[frontier] = True
        if processed[n - 1]:
            break
        nxt = set()
        for v in frontier:
            for m in tgt[src == v]:
                if not processed[m]:
                    nxt.add(int(m))
        frontier = sorted(nxt)
        assert frontier, "last neuron unreachable"
    return in_lists, waves


def _cone(src, tgt):
    """Returns (steps, fc_live).

    steps: ordered list of (node, [(srckey, channel), ...]) where srckey is
      'x' for the image input or an int neuron id computed in an earlier step.
    fc_live: [(channel_of_255, src_node), ...] live channels of the readout.
    """
    n = N_NEURONS
    in_lists, waves = _schedule(src, tgt)
    wave_of = {}
    for wi, w in enumerate(waves):
        for v in w:
            if v not in wave_of:
                wave_of[v] = wi
    BIG = 1 << 30
    w255 = wave_of[n - 1]
    fc_live = [(c, int(s)) for c, s in enumerate(in_lists[n - 1])
               if wave_of.get(int(s), BIG) < w255]

    live = {}
    stack = [s for _, s in fc_live]
    seen = set()
    while stack:
        v = stack.pop()
        if v in seen:
            continue
        seen.add(v)
        if v == 0:
            live[0] = [('x', 0)]
            continue
        chans = [(int(s), c) for c, s in enumerate(in_lists[v])
                 if wave_of.get(int(s), BIG) < wave_of[v]]
        assert chans, f"cone node {v} has no live channels"
        live[v] = [(s, c) for s, c in chans]
        stack += [s for s, _ in chans]

    steps = sorted(live.items(), key=lambda kv: wave_of[kv[0]])
    return steps, fc_live


# ---------------------------------------------------------- host-side packing
def _toeplitz(w):
    """w [5,5] -> [160, 28] banded matrix over K=(dy, row).

    Slot row r of each 32-row group holds padded-image column (r+2) mod 32,
    so the activation value at x lands at row x (32-aligned writes; wrapped
    rows 28..31 hold the zero x-padding)."""
    T = np.zeros((160, HW), np.float32)
    for dy in range(5):
        for dx in range(5):
            for xc in range(HW):
                T[dy * 32 + (xc + dx - 2) % 32, xc] = w[dy, dx]
    return T


def _xstack(xb):
    """xb [B,28,28] -> [160, B*32] fp16: five y-shifted padded slot copies.

    Slot_dy[r, b*32+yp] = xpad[b, yp+dy-2, (r+2) % 32]."""
    xpad = np.zeros((B, 32, 32), np.float32)
    xpad[:, 2:30, 2:30] = xb
    st = np.zeros((5, 32, B, 32), np.float32)
    for dy in range(5):
        lo, hi = max(0, 2 - dy), min(32, 34 - dy)
        st[dy, :, :, lo:hi] = xpad[:, lo + dy - 2:hi + dy - 2, :].transpose(2, 0, 1)
    st = np.roll(st, -2, axis=1)
    return st.reshape(160, B * 32).astype(np.float16)


def _pack(steps, fc_live, conv_w, conv_b, fc1_w, fc1_b, fc2_w, fc2_b):
    """Builds mainh (f16, [128, *]), tb (f16, [33, *]), consts (f32,
    [128, 36]), f1w (f16, [128, 1400*nfc]) and the slot map."""
    slots = {}
    col = 0
    for v, chans in steps:
        for j, _ in enumerate(chans):
            slots[('toep', v, j)] = col
            col += HW
    a_cols = col
    slots['xs'] = a_cols
    n_toep = a_cols  # same column budget in tb (B parts align with A parts)

    mainh = np.zeros((128, a_cols + 512), np.float16)
    tb = np.zeros((32, a_cols + 512), np.float16)
    for v, chans in steps:
        for j, (skey, ch) in enumerate(chans):
            T = _toeplitz(conv_w[v, 0, ch])
            c0 = slots[('toep', v, j)]
            mainh[:, c0:c0 + HW] = T[:128]
            tb[:, c0:c0 + HW] = T[128:]

    # consts: identity (16x16), fc2wA [128,10], fc2wB [96,10] w/ bias row;
    # conv biases ride a separate tiny tensor so they land early
    nsteps = len(steps)
    consts = np.zeros((128, 36), np.float32)
    consts[:B, 0:B] = np.eye(B, dtype=np.float32)
    w2t = fc2_w.T  # [200, 10]
    consts[:, 16:26] = w2t[:128]
    consts[:FC_HID - 128, 26:36] = w2t[128:]
    consts[72, 26:36] = fc2_b
    # padded to 32 rows: descriptor generation has a fast path only for
    # multiple-of-16 partition counts
    cbt = np.zeros((32, nsteps), np.float32)
    for i, (v, _) in enumerate(steps):
        slots[('cb', v)] = i
        cbt[:HW, i] = conv_b[v]

    n_fc = len(fc_live)
    f1p = np.zeros((128, 1400 * n_fc), np.float16)
    for k, (c, s) in enumerate(fc_live):
        blk = fc1_w[:, c * 784:(c + 1) * 784].reshape(FC_HID, HW, HW)  # [h, y, x]
        arr = blk.reshape(FC_HID, 4, 7, HW).transpose(1, 3, 2, 0)  # [yg, x, ysub, h]
        f1p[:, k * 1400:(k + 1) * 1400] = np.pad(
            arr, ((0, 0), (0, 4), (0, 0), (0, 0))).reshape(128, 1400)
    f1p[28, 0:FC_HID] = fc1_b  # vs ones-row of fc stack (k=0, sj=0 only)
    return mainh, tb, consts, cbt, f1p, slots


# ---------------------------------------------------------- device program
def _build(steps, fc_live, ncolsA, nfc):
    nc = _Bacc("TRN2", target_bir_lowering=False)
    nsteps = len(steps)
    mainh_d = nc.dram_tensor("mainh", [128, ncolsA + 512], F16, kind="ExternalInput")
    tb_d = nc.dram_tensor("tb", [32, ncolsA + 512], F16, kind="ExternalInput")
    consts_d = nc.dram_tensor("consts", [128, 36], F32, kind="ExternalInput")
    cbt_d = nc.dram_tensor("cbt", [32, nsteps], F32, kind="ExternalInput")
    f1w_d = nc.dram_tensor("f1w", [128, 1400 * nfc], F16, kind="ExternalInput")
    out_d = nc.dram_tensor("out", [B, N_CLS], F32, kind="ExternalOutput")

    feeds_conv = set()
    for v, chans in steps:
        for skey, _ in chans:
            if skey != 'x':
                feeds_conv.add(skey)
    fc_srcs = [s for _, s in fc_live]
    SL = _SLOTS
    YS = 12  # relu y-split point: ACT gets [0, YS), DVE [YS, 28)

    with tile.TileContext(nc) as tc:
        with (
            tc.tile_pool(name="persist", bufs=1) as pool,
            tc.tile_pool(name="cpsum", bufs=1, space="PSUM") as cpp,
            tc.tile_pool(name="qpsum", bufs=1, space="PSUM") as qpp,
            tc.tile_pool(name="fpsum", bufs=1, space="PSUM") as fpp,
        ):
            mainh = pool.tile([128, ncolsA + 512], F16, tag="mainh")
            tbt = pool.tile([32, ncolsA + 512], F16, tag="tb")
            consts = pool.tile([128, 36], F32, tag="consts")
            cbt = pool.tile([32, nsteps], F32, tag="cbt")
            f1w = pool.tile([128, 1400 * nfc], F16, tag="f1w")
            # mainh gates conv0; f1w is not needed until fc1 -- both on the
            # HWDGE sync queue (mainh first), small tensors on scalar.
            nc.sync.dma_start(mainh[:], mainh_d[:])
            nc.sync.dma_start(f1w[:], f1w_d[:])
            nc.scalar.dma_start(tbt[:], tb_d[:])
            nc.scalar.dma_start(consts[:], consts_d[:])
            nc.gpsimd.dma_start(cbt[:], cbt_d[:])

            # dummy activation: hoists the single ACT table load to the top
            swu = pool.tile([1, 2], F32, tag="swu")
            nc.gpsimd.memset(swu[:], 1.0)
            nc.scalar.activation(swu[:, 0:1], swu[:, 0:1], AF.Exp)

            # activation slot tiles per conv producer (fp16, zero borders);
            # memsets split DVE/GPSIMD so neither engine's prologue is long
            stacks = {}
            ms_engines = [nc.vector, nc.gpsimd]
            for mi, v in enumerate(sorted(feeds_conv)):
                eng = ms_engines[mi % 2]
                a = pool.tile([128, B * 32], F16, name=f"stA_{v}", tag=f"stA_{v}")
                b = pool.tile([32, B * 32], F16, name=f"stB_{v}", tag=f"stB_{v}")
                eng.memset(a[:], 0.0)
                eng.memset(b[:], 0.0)
                stacks[v] = (a, b)
            fcstacks = {}
            for sv in sorted(set(fc_srcs)):
                t = pool.tile([128, B * 7], F16, name=f"fcst_{sv}", tag=f"fcst_{sv}")
                nc.gpsimd.memset(t[:], 0.0)
                fv = t[:].rearrange("p (b s) -> p b s", s=7)
                # ones-row for the fc1 bias trick; partition starts must be
                # 32-aligned, so write rows 0:32 -- rows 0:28 are overwritten
                # by the g=0 quarter write, rows 29:31 face zero-padded
                # stationary rows
                nc.gpsimd.memset(fv[0:32, :, 0:1], 1.0)
                fcstacks[sv] = t
            h2 = pool.tile([96, B], F32, tag="h2")
            nc.gpsimd.memset(h2[64:96, :], 1.0)

            xsa = mainh[:, SL['xs']:SL['xs'] + 512]
            xsb = tbt[:, SL['xs']:SL['xs'] + 512]

            def slot_slices(key):
                if key == 'x':
                    av, bv = xsa, xsb
                else:
                    a, b = stacks[key]
                    av, bv = a[:], b[:]
                return (av.rearrange("p (b y) -> p b y", y=32),
                        bv.rearrange("p (b y) -> p b y", y=32))

            # --- conv chain ---
            for v, chans in steps:
                nch = len(chans)
                cb = cbt[:HW, SL[('cb', v)]:SL[('cb', v)] + 1]
                fc_only = v in fcstacks and v not in feeds_conv

                if fc_only:
                    # four independent quarter PSUMs: each regroup write has
                    # its own tile, so the DVE/ACT pairs truly overlap
                    fst = fcstacks[v]
                    fv = fst[:].rearrange("p (b s) -> p b s", s=7)
                    # four PSUM tiles in four banks: concurrently-open
                    # accumulation groups must not share a PSUM bank, and
                    # separate tiles keep the regroup writes parallel
                    qs = [qpp.tile([HW, B * 7], F32, tag=f"q{g}",
                                   name=f"q{v}_{g}")[:]
                          for g in range(4)]
                    for j, (skey, ch) in enumerate(chans):
                        c0 = SL[('toep', v, j)]
                        av, bv = slot_slices(skey)
                        for g in range(4):
                            ysl = slice(2 + 7 * g, 9 + 7 * g)
                            nc.tensor.matmul(qs[g], tbt[:, c0:c0 + HW],
                                             bv[:, :, ysl],
                                             start=(j == 0), stop=False)
                        for g in range(4):
                            ysl = slice(2 + 7 * g, 9 + 7 * g)
                            nc.tensor.matmul(qs[g], mainh[:, c0:c0 + HW],
                                             av[:, :, ysl],
                                             start=False, stop=(j == nch - 1))
                    for g in range(4):
                        dst = fv[g * 32:g * 32 + HW, :, :]
                        qv = qs[g].rearrange("p (b y) -> p b y", y=7)
                        if g % 2 == 0:
                            nc.vector.tensor_scalar(dst, qv, cb, 0.0,
                                                    ALU.add, ALU.max)
                        else:
                            nc.scalar.activation(dst, qv, AF.Relu, bias=cb,
                                                 scale=1.0)
                    continue

                # y-split PSUM halves: ACT relus the low half while DVE does
                # the high half -- separate tiles, so no reader serialization
                plo = cpp.tile([HW, B * YS], F32, tag="pslo", name=f"plo{v}")
                phi = cpp.tile([HW, B * (HW - YS)], F32, tag="pshi",
                               name=f"phi{v}")
                for j, (skey, ch) in enumerate(chans):
                    c0 = SL[('toep', v, j)]
                    av, bv = slot_slices(skey)
                    # B part first: its moving operand is copied first, so
                    # the PE can start before the dy slot copies finish
                    nc.tensor.matmul(plo[:], tbt[:, c0:c0 + HW],
                                     bv[:, :, 2:2 + YS],
                                     start=(j == 0), stop=False)
                    nc.tensor.matmul(phi[:], tbt[:, c0:c0 + HW],
                                     bv[:, :, 2 + YS:30],
                                     start=(j == 0), stop=False)
                    nc.tensor.matmul(plo[:], mainh[:, c0:c0 + HW],
                                     av[:, :, 2:2 + YS],
                                     start=False, stop=(j == nch - 1))
                    nc.tensor.matmul(phi[:], mainh[:, c0:c0 + HW],
                                     av[:, :, 2 + YS:30],
                                     start=False, stop=(j == nch - 1))
                plov = plo[:].rearrange("p (b y) -> p b y", y=YS)
                phiv = phi[:].rearrange("p (b y) -> p b y", y=HW - YS)

                av, bv = slot_slices(v)
                g2 = av[64:64 + HW, :, 2:30]
                nc.scalar.activation(av[64:64 + HW, :, 2:2 + YS], plov,
                                     AF.Relu, bias=cb, scale=1.0)
                nc.vector.tensor_scalar(av[64:64 + HW, :, 2 + YS:30], phiv,
                                        cb, 0.0, ALU.add, ALU.max)
                nc.vector.tensor_copy(bv[0:HW, :, 0:28], g2)
                nc.vector.tensor_copy(av[0:HW, :, 4:32], g2)
                nc.vector.tensor_copy(av[32:32 + HW, :, 3:31], g2)
                nc.scalar.copy(av[96:96 + HW, :, 1:29], g2)
                if v in fcstacks:  # node feeds both conv and fc (rare)
                    fst = fcstacks[v]
                    fv = fst[:].rearrange("p (b s) -> p b s", s=7)
                    for g in range(4):
                        dst = fv[g * 32:g * 32 + HW, :, :]
                        lo, hi = 7 * g, 7 * g + 7
                        if hi <= YS:
                            src3 = plov[:, :, lo:hi]
                        elif lo >= YS:
                            src3 = phiv[:, :, lo - YS:hi - YS]
                        else:
                            nc.vector.tensor_scalar(dst[:, :, 0:YS - lo],
                                                    plov[:, :, lo:YS], cb,
                                                    0.0, ALU.add, ALU.max)
                            nc.vector.tensor_scalar(dst[:, :, YS - lo:],
                                                    phiv[:, :, 0:hi - YS], cb,
                                                    0.0, ALU.add, ALU.max)
                            continue
                        nc.vector.tensor_scalar(dst, src3, cb, 0.0,
                                                ALU.add, ALU.max)

            # --- fc1: activations stationary, hidden units streamed ---
            p1 = fpp.tile([B, FC_HID], F32, tag="p1")
            for k in range(nfc):
                fst = fcstacks[fc_live[k][1]]
                fv = fst[:].rearrange("p (b s) -> p b s", s=7)
                for sj in range(7):
                    i = k * 7 + sj
                    nc.tensor.matmul(p1[:], fv[:, :, sj:sj + 1],
                                     f1w[:, (k * 7 + sj) * 200:(k * 7 + sj + 1) * 200],
                                     start=(i == 0), stop=(i == 7 * nfc - 1))
            # bias already accumulated; relu+copy in one DVE op
            ht = pool.tile([B, FC_HID], F32, tag="ht")
            nc.vector.tensor_scalar_max(ht[:], p1[:], 0.0)
            idn = consts[:B, 0:B]
            # t1/t2/ps2 share one PSUM bank: their matmul groups never
            # overlap in time (transposes close before fc2 starts)
            t12 = fpp.tile([128, 2 * B + N_CLS], F32, tag="t12")
            nc.tensor.transpose(t12[:, 0:B], ht[:, 0:128], idn)
            nc.tensor.transpose(t12[0:FC_HID - 128, B:2 * B],
                                ht[:, 128:FC_HID], idn)
            h1 = pool.tile([128, B], F32, tag="h1")
            nc.vector.tensor_copy(h1[:], t12[:, 0:B])
            nc.scalar.copy(h2[0:FC_HID - 128, :],
                           t12[0:FC_HID - 128, B:2 * B])

            # --- fc2 (hidden stationary -> logits [b, cls]) + log_softmax ---
            ps2 = t12[0:B, 2 * B:2 * B + N_CLS]
            nc.tensor.matmul(ps2, h1[:], consts[:, 16:26],
                             start=True, stop=False)
            nc.tensor.matmul(ps2, h2[:], consts[0:96, 26:36],
                             start=False, stop=True)
            ex = pool.tile([B, N_CLS], F32, tag="ex")
            nc.scalar.activation(ex[:], ps2, AF.Exp)
            sm = pool.tile([B, 1], F32, tag="sm")
            nc.vector.reduce_sum(sm[:], ex[:], axis=mybir.AxisListType.X)
            lse = pool.tile([B, 1], F32, tag="lse")
            nc.scalar.activation(lse[:], sm[:], AF.Ln)
            res = pool.tile([B, N_CLS], F32, tag="res")
            nc.vector.tensor_scalar_sub(res[:], ps2, lse[:])
            nc.sync.dma_start(out_d[:], res[:])
    nc.compile()
    return nc


_SLOTS = None
_PROG_CACHE = {}


def kernel(x, src, tgt, conv_w, conv_b, fc1_w, fc1_b, fc2_w, fc2_b):
    global _SLOTS, LAST_RESULT
    x = np.asarray(x, np.float32)
    src = np.asarray(src, np.int32)
    tgt = np.asarray(tgt, np.int32)
    conv_w = np.asarray(conv_w, np.float32)
    conv_b = np.asarray(conv_b, np.float32)
    fc1_w = np.asarray(fc1_w, np.float32)
    fc1_b = np.asarray(fc1_b, np.float32)
    fc2_w = np.asarray(fc2_w, np.float32)
    fc2_b = np.asarray(fc2_b, np.float32)

    steps, fc_live = _cone(src, tgt)
    mainh0, tb, consts, cbt, f1p, slots = _pack(steps, fc_live, conv_w, conv_b,
                                                fc1_w, fc1_b, fc2_w, fc2_b)
    _SLOTS = slots
    ncolsA = slots['xs']

    key = (tuple((v, tuple(ch)) for v, ch in steps), tuple(fc_live), ncolsA)
    if key not in _PROG_CACHE:
        _PROG_CACHE[key] = _build(steps, fc_live, ncolsA, len(fc_live))
    nc = _PROG_CACHE[key]

    xs = x[:, 0]  # [128, 28, 28]
    in_maps = []
    for c in range(N_CORES):
        st = _xstack(xs[c * B:(c + 1) * B])
        mainh = mainh0.copy()
        mainh[:, ncolsA:ncolsA + 512] = st[:128]
        tbc = tb.copy()
        tbc[:, ncolsA:ncolsA + 512] = st[128:160]
        in_maps.append({"mainh": mainh, "tb": tbc, "consts": consts,
                        "cbt": cbt, "f1w": f1p})

    LAST_RESULT = run_bass_kernel_spmd(nc, in_maps, list(range(N_CORES)))
    out = np.concatenate([r["out"] for r in LAST_RESULT.results], axis=0)
    return out.astype(np.float32)


# revision 15
# speedup vs baseline: 1.1670x; 1.0241x over previous
"""Trainium2 Bass kernel for nn_Net_21852793602541 (gnn_message_passing).

The reference net's output depends only on a tiny dependency cone of the
message-passing graph: the final hidden layer reads the wave-2 snapshot of
neuron activations, so only neurons feeding neuron 255 through channels whose
source was already processed matter.  For the fixed graph that is a 3-conv
chain (x -> n0 -> n172 -> n215), one 784->200 FC block, a 200->10 FC and
log_softmax.  The cone is recomputed at runtime from the src/tgt inputs.

Per-core mapping (data-parallel over batch, 16 images/core on 8 cores):
  * 5x5 conv == one PE accumulation group: contraction K = (dy, slot-row)
    with a banded-Toeplitz stationary (fp16) against 5 y-shifted slot copies
    of the padded image block (fp16), N = (batch, y) = 448.  The B-part
    stationary carries a 33rd row holding the conv bias against an all-ones
    row of the moving operand, so PSUM already contains the bias and the
    relu is a plain max(0, x).
  * the relu is split PSUM-side between ACT and DVE (y halves); the slot
    replication copies are spread over DVE (3) and GPSIMD (1); the B-part
    matmul of the next conv is issued first so it can start on the earliest
    copy.
  * fc1 streams the 200 hidden units as the moving operand (7 accumulated
    matmuls); fc1 bias rides a ones-row of the fc stack.  The [16, 200]
    result is relu'd on DVE, transposed via PE, and fc2 uses the hidden
    vectors as the *stationary* so logits land [batch, cls] with no final
    transpose; fc2 bias rides a ones-row of the second hidden chunk.
  * log_softmax without max-subtraction (logits are O(5)): ACT exp, DVE
    reduce, ACT ln, one fused DVE subtract.
  * a Bacc subclass pins the activation-table pass to the
    natural_log_exp_and_others set (covers Relu/Identity/Exp/Ln/Copy), so
    exactly one ACT_TABLE_LOAD runs, early and off the critical path.
"""

import numpy as np

import concourse.bass as bass
import concourse.tile as tile
from concourse import bacc, mybir
from concourse.bacc import _bass_rust
from concourse.bass_utils import run_bass_kernel_spmd
from concourse.hw_specs import get_activation_tables

# The axon NTFF profile hook normally lives in antenv.axon_hooks, which this
# image lacks.  Shim it from the boot module's ctypes implementation so
# BASS_TRACE=1 profiling works; degrade silently if unavailable.
try:
    import antenv.axon_hooks  # noqa: F401
except ImportError:
    try:
        import sys as _sys
        import types as _types

        from trn_agent_boot.trn_boot import _ntff_profile_via_ctypes

        _hook = _ntff_profile_via_ctypes('/opt/axon/libaxon_pjrt.so')
        _mod = _types.ModuleType('antenv.axon_hooks')
        _mod.get_axon_ntff_profile_hook = lambda: _hook
        _mod.set_axon_ntff_profile_hook = lambda h: None
        _sys.modules['antenv.axon_hooks'] = _mod
    except Exception:
        pass

F32 = mybir.dt.float32
F16 = mybir.dt.float16
AF = mybir.ActivationFunctionType
ALU = mybir.AluOpType
N_NEURONS = 256
N_CORES = 8
B_TOTAL = 128
B = B_TOTAL // N_CORES  # 16 images per core
HW = 28
FC_HID = 200
N_CLS = 10

_ACT_SET = 'natural_log_exp_and_others'

LAST_RESULT = None  # BassKernelResults of the most recent run (for profiling)


class _Bacc(bacc.Bacc):
    """Bacc whose activation-table pass only ever picks the combined
    exp+ln set, so the whole kernel needs a single ACT_TABLE_LOAD (the
    greedy first-match pass would otherwise load exp_and_others early and
    natural_log right between the tail's Exp and Ln)."""

    def insert_act_table_loads(self):
        has_activation = any(
            isinstance(i, mybir.InstActivation)
            for b in self.main_func.blocks
            for i in b.instructions
        )
        if not has_activation:
            return
        tables = get_activation_tables(self.m.arch)
        filt = [(name, (fns if name == _ACT_SET else set()))
                for name, fns in tables.items()]
        _bass_rust.insert_act_table_loads(self, filt)


# ---------------------------------------------------------------- schedule
def _schedule(src, tgt):
    n = N_NEURONS
    in_lists = [src[np.where(tgt == i)[0]].astype(np.int64).tolist() for i in range(n)]
    waves = []
    processed = np.zeros(n, bool)
    frontier = [0]
    while True:
        waves.append(list(frontier))
        processed[frontier] = True
        if processed[n - 1]:
            break
        nxt = set()
        for v in frontier:
            for m in tgt[src == v]:
                if not processed[m]:
                    nxt.add(int(m))
        frontier = sorted(nxt)
        assert frontier, "last neuron unreachable"
    return in_lists, waves


def _cone(src, tgt):
    """Returns (steps, fc_live).

    steps: ordered list of (node, [(srckey, channel), ...]) where srckey is
      'x' for the image input or an int neuron id computed in an earlier step.
    fc_live: [(channel_of_255, src_node), ...] live channels of the readout.
    """
    n = N_NEURONS
    in_lists, waves = _schedule(src, tgt)
    wave_of = {}
    for wi, w in enumerate(waves):
        for v in w:
            if v not in wave_of:
                wave_of[v] = wi
    BIG = 1 << 30
    w255 = wave_of[n - 1]
    fc_live = [(c, int(s)) for c, s in enumerate(in_lists[n - 1])
               if wave_of.get(int(s), BIG) < w255]

    live = {}
    stack = [s for _, s in fc_live]
    seen = set()
    while stack:
        v = stack.pop()
        if v in seen:
            continue
        seen.add(v)
        if v == 0:
            live[0] = [('x', 0)]
            continue
        chans = [(int(s), c) for c, s in enumerate(in_lists[v])
                 if wave_of.get(int(s), BIG) < wave_of[v]]
        assert chans, f"cone node {v} has no live channels"
        live[v] = [(s, c) for s, c in chans]
        stack += [s for s, _ in chans]

    steps = sorted(live.items(), key=lambda kv: wave_of[kv[0]])
    return steps, fc_live


# ---------------------------------------------------------- host-side packing
def _toeplitz(w):
    """w [5,5] -> [160, 28] banded matrix over K=(dy, row).

    Slot row r of each 32-row group holds padded-image column (r+2) mod 32,
    so the activation value at x lands at row x (32-aligned writes; wrapped
    rows 28..31 hold the zero x-padding)."""
    T = np.zeros((160, HW), np.float32)
    for dy in range(5):
        for dx in range(5):
            for xc in range(HW):
                T[dy * 32 + (xc + dx - 2) % 32, xc] = w[dy, dx]
    return T


def _xstack(xb):
    """xb [B,28,28] -> [160, B*32] fp16: five y-shifted padded slot copies.

    Slot_dy[r, b*32+yp] = xpad[b, yp+dy-2, (r+2) % 32]."""
    xpad = np.zeros((B, 32, 32), np.float32)
    xpad[:, 2:30, 2:30] = xb
    st = np.zeros((5, 32, B, 32), np.float32)
    for dy in range(5):
        lo, hi = max(0, 2 - dy), min(32, 34 - dy)
        st[dy, :, :, lo:hi] = xpad[:, lo + dy - 2:hi + dy - 2, :].transpose(2, 0, 1)
    st = np.roll(st, -2, axis=1)
    return st.reshape(160, B * 32).astype(np.float16)


def _pack(steps, fc_live, conv_w, conv_b, fc1_w, fc1_b, fc2_w, fc2_b):
    """Builds mainh (f16, [128, *]), tb (f16, [33, *]), consts (f32,
    [128, 36]), f1w (f16, [128, 1400*nfc]) and the slot map."""
    slots = {}
    col = 0
    for v, chans in steps:
        for j, _ in enumerate(chans):
            slots[('toep', v, j)] = col
            col += HW
    a_cols = col
    slots['xs'] = a_cols
    n_toep = a_cols  # same column budget in tb (B parts align with A parts)

    mainh = np.zeros((128, a_cols + 512), np.float16)
    tb = np.zeros((32, a_cols + 512), np.float16)
    for v, chans in steps:
        for j, (skey, ch) in enumerate(chans):
            T = _toeplitz(conv_w[v, 0, ch])
            c0 = slots[('toep', v, j)]
            mainh[:, c0:c0 + HW] = T[:128]
            tb[:, c0:c0 + HW] = T[128:]

    # consts: identity (16x16), fc2wA [128,10], fc2wB [96,10] w/ bias row;
    # conv biases ride a separate tiny tensor so they land early
    nsteps = len(steps)
    consts = np.zeros((128, 36), np.float32)
    consts[:B, 0:B] = np.eye(B, dtype=np.float32)
    w2t = fc2_w.T  # [200, 10]
    consts[:, 16:26] = w2t[:128]
    consts[:FC_HID - 128, 26:36] = w2t[128:]
    consts[72, 26:36] = fc2_b
    # padded to 32 rows: descriptor generation has a fast path only for
    # multiple-of-16 partition counts
    cbt = np.zeros((32, nsteps), np.float32)
    for i, (v, _) in enumerate(steps):
        slots[('cb', v)] = i
        cbt[:HW, i] = conv_b[v]

    n_fc = len(fc_live)
    f1p = np.zeros((128, 1400 * n_fc), np.float16)
    for k, (c, s) in enumerate(fc_live):
        blk = fc1_w[:, c * 784:(c + 1) * 784].reshape(FC_HID, HW, HW)  # [h, y, x]
        arr = blk.reshape(FC_HID, 4, 7, HW).transpose(1, 3, 2, 0)  # [yg, x, ysub, h]
        f1p[:, k * 1400:(k + 1) * 1400] = np.pad(
            arr, ((0, 0), (0, 4), (0, 0), (0, 0))).reshape(128, 1400)
    f1p[28, 0:FC_HID] = fc1_b  # vs ones-row of fc stack (k=0, sj=0 only)
    return mainh, tb, consts, cbt, f1p, slots


# ---------------------------------------------------------- device program
def _build(steps, fc_live, ncolsA, nfc):
    nc = _Bacc("TRN2", target_bir_lowering=False)
    nsteps = len(steps)
    mainh_d = nc.dram_tensor("mainh", [128, ncolsA + 512], F16, kind="ExternalInput")
    tb_d = nc.dram_tensor("tb", [32, ncolsA + 512], F16, kind="ExternalInput")
    consts_d = nc.dram_tensor("consts", [128, 36], F32, kind="ExternalInput")
    cbt_d = nc.dram_tensor("cbt", [32, nsteps], F32, kind="ExternalInput")
    f1w_d = nc.dram_tensor("f1w", [128, 1400 * nfc], F16, kind="ExternalInput")
    out_d = nc.dram_tensor("out", [B, N_CLS], F32, kind="ExternalOutput")

    feeds_conv = set()
    for v, chans in steps:
        for skey, _ in chans:
            if skey != 'x':
                feeds_conv.add(skey)
    fc_srcs = [s for _, s in fc_live]
    SL = _SLOTS
    YS = 14  # relu y-split point: ACT gets [0, YS), DVE [YS, 28)

    with tile.TileContext(nc) as tc:
        with (
            tc.tile_pool(name="persist", bufs=1) as pool,
            tc.tile_pool(name="cpsum", bufs=1, space="PSUM") as cpp,
            tc.tile_pool(name="qpsum", bufs=1, space="PSUM") as qpp,
            tc.tile_pool(name="fpsum", bufs=1, space="PSUM") as fpp,
        ):
            mainh = pool.tile([128, ncolsA + 512], F16, tag="mainh")
            tbt = pool.tile([32, ncolsA + 512], F16, tag="tb")
            consts = pool.tile([128, 36], F32, tag="consts")
            cbt = pool.tile([32, nsteps], F32, tag="cbt")
            f1w = pool.tile([128, 1400 * nfc], F16, tag="f1w")
            # mainh gates conv0; f1w is not needed until fc1 -- both on the
            # HWDGE sync queue (mainh first), small tensors on scalar.
            nc.sync.dma_start(mainh[:], mainh_d[:])
            nc.sync.dma_start(f1w[:], f1w_d[:])
            nc.scalar.dma_start(tbt[:], tb_d[:])
            nc.scalar.dma_start(consts[:], consts_d[:])
            nc.gpsimd.dma_start(cbt[:], cbt_d[:])

            # activation slot tiles per conv producer (fp16, zero borders);
            # memsets split DVE/GPSIMD so neither engine's prologue is long
            stacks = {}
            ms_engines = [nc.vector, nc.gpsimd]
            for mi, v in enumerate(sorted(feeds_conv)):
                eng = ms_engines[mi % 2]
                a = pool.tile([128, B * 32], F16, name=f"stA_{v}", tag=f"stA_{v}")
                b = pool.tile([32, B * 32], F16, name=f"stB_{v}", tag=f"stB_{v}")
                eng.memset(a[:], 0.0)
                eng.memset(b[:], 0.0)
                stacks[v] = (a, b)
            fcstacks = {}
            for sv in sorted(set(fc_srcs)):
                t = pool.tile([128, B * 7], F16, name=f"fcst_{sv}", tag=f"fcst_{sv}")
                nc.gpsimd.memset(t[:], 0.0)
                fv = t[:].rearrange("p (b s) -> p b s", s=7)
                # ones-row for the fc1 bias trick; partition starts must be
                # 32-aligned, so write rows 0:32 -- rows 0:28 are overwritten
                # by the g=0 quarter write, rows 29:31 face zero-padded
                # stationary rows
                nc.gpsimd.memset(fv[0:32, :, 0:1], 1.0)
                fcstacks[sv] = t
            h2 = pool.tile([96, B], F32, tag="h2")
            nc.gpsimd.memset(h2[64:96, :], 1.0)

            xsa = mainh[:, SL['xs']:SL['xs'] + 512]
            xsb = tbt[:, SL['xs']:SL['xs'] + 512]

            def slot_slices(key):
                if key == 'x':
                    av, bv = xsa, xsb
                else:
                    a, b = stacks[key]
                    av, bv = a[:], b[:]
                return (av.rearrange("p (b y) -> p b y", y=32),
                        bv.rearrange("p (b y) -> p b y", y=32))

            # --- conv chain ---
            for v, chans in steps:
                nch = len(chans)
                cb = cbt[:HW, SL[('cb', v)]:SL[('cb', v)] + 1]
                fc_only = v in fcstacks and v not in feeds_conv

                if fc_only:
                    # four independent quarter PSUMs: each regroup write has
                    # its own tile, so the DVE/ACT pairs truly overlap
                    fst = fcstacks[v]
                    fv = fst[:].rearrange("p (b s) -> p b s", s=7)
                    # four PSUM tiles in four banks: concurrently-open
                    # accumulation groups must not share a PSUM bank, and
                    # separate tiles keep the regroup writes parallel
                    qs = [qpp.tile([HW, B * 7], F32, tag=f"q{g}",
                                   name=f"q{v}_{g}")[:]
                          for g in range(4)]
                    for j, (skey, ch) in enumerate(chans):
                        c0 = SL[('toep', v, j)]
                        av, bv = slot_slices(skey)
                        for g in range(4):
                            ysl = slice(2 + 7 * g, 9 + 7 * g)
                            nc.tensor.matmul(qs[g], tbt[:, c0:c0 + HW],
                                             bv[:, :, ysl],
                                             start=(j == 0), stop=False)
                        for g in range(4):
                            ysl = slice(2 + 7 * g, 9 + 7 * g)
                            nc.tensor.matmul(qs[g], mainh[:, c0:c0 + HW],
                                             av[:, :, ysl],
                                             start=False, stop=(j == nch - 1))
                    for g in range(4):
                        dst = fv[g * 32:g * 32 + HW, :, :]
                        qv = qs[g].rearrange("p (b y) -> p b y", y=7)
                        if g % 2 == 0:
                            nc.vector.tensor_scalar(dst, qv, cb, 0.0,
                                                    ALU.add, ALU.max)
                        else:
                            nc.scalar.activation(dst, qv, AF.Relu, bias=cb,
                                                 scale=1.0)
                    continue

                # y-split PSUM halves: ACT relus the low half while DVE does
                # the high half -- separate tiles, so no reader serialization
                plo = cpp.tile([HW, B * YS], F32, tag="pslo", name=f"plo{v}")
                phi = cpp.tile([HW, B * (HW - YS)], F32, tag="pshi",
                               name=f"phi{v}")
                for j, (skey, ch) in enumerate(chans):
                    c0 = SL[('toep', v, j)]
                    av, bv = slot_slices(skey)
                    # B part first: its moving operand is copied first, so
                    # the PE can start before the dy slot copies finish
                    nc.tensor.matmul(plo[:], tbt[:, c0:c0 + HW],
                                     bv[:, :, 2:2 + YS],
                                     start=(j == 0), stop=False)
                    nc.tensor.matmul(phi[:], tbt[:, c0:c0 + HW],
                                     bv[:, :, 2 + YS:30],
                                     start=(j == 0), stop=False)
                    nc.tensor.matmul(plo[:], mainh[:, c0:c0 + HW],
                                     av[:, :, 2:2 + YS],
                                     start=False, stop=(j == nch - 1))
                    nc.tensor.matmul(phi[:], mainh[:, c0:c0 + HW],
                                     av[:, :, 2 + YS:30],
                                     start=False, stop=(j == nch - 1))
                plov = plo[:].rearrange("p (b y) -> p b y", y=YS)
                phiv = phi[:].rearrange("p (b y) -> p b y", y=HW - YS)

                av, bv = slot_slices(v)
                g2 = av[64:64 + HW, :, 2:30]
                nc.scalar.activation(av[64:64 + HW, :, 2:2 + YS], plov,
                                     AF.Relu, bias=cb, scale=1.0)
                nc.vector.tensor_scalar(av[64:64 + HW, :, 2 + YS:30], phiv,
                                        cb, 0.0, ALU.add, ALU.max)
                nc.vector.tensor_copy(bv[0:HW, :, 0:28], g2)
                nc.vector.tensor_copy(av[0:HW, :, 4:32], g2)
                nc.vector.tensor_copy(av[32:32 + HW, :, 3:31], g2)
                nc.scalar.copy(av[96:96 + HW, :, 1:29], g2)
                if v in fcstacks:  # node feeds both conv and fc (rare)
                    fst = fcstacks[v]
                    fv = fst[:].rearrange("p (b s) -> p b s", s=7)
                    for g in range(4):
                        dst = fv[g * 32:g * 32 + HW, :, :]
                        lo, hi = 7 * g, 7 * g + 7
                        if hi <= YS:
                            src3 = plov[:, :, lo:hi]
                        elif lo >= YS:
                            src3 = phiv[:, :, lo - YS:hi - YS]
                        else:
                            nc.vector.tensor_scalar(dst[:, :, 0:YS - lo],
                                                    plov[:, :, lo:YS], cb,
                                                    0.0, ALU.add, ALU.max)
                            nc.vector.tensor_scalar(dst[:, :, YS - lo:],
                                                    phiv[:, :, 0:hi - YS], cb,
                                                    0.0, ALU.add, ALU.max)
                            continue
                        nc.vector.tensor_scalar(dst, src3, cb, 0.0,
                                                ALU.add, ALU.max)

            # --- fc1: activations stationary, hidden units streamed ---
            p1 = fpp.tile([B, FC_HID], F32, tag="p1")
            for k in range(nfc):
                fst = fcstacks[fc_live[k][1]]
                fv = fst[:].rearrange("p (b s) -> p b s", s=7)
                for sj in range(7):
                    i = k * 7 + sj
                    nc.tensor.matmul(p1[:], fv[:, :, sj:sj + 1],
                                     f1w[:, (k * 7 + sj) * 200:(k * 7 + sj + 1) * 200],
                                     start=(i == 0), stop=(i == 7 * nfc - 1))
            # bias already accumulated; relu+copy in one DVE op
            ht = pool.tile([B, FC_HID], F32, tag="ht")
            nc.vector.tensor_scalar_max(ht[:], p1[:], 0.0)
            idn = consts[:B, 0:B]
            # t1/t2/ps2 share one PSUM bank: their matmul groups never
            # overlap in time (transposes close before fc2 starts)
            t12 = fpp.tile([128, 2 * B + N_CLS], F32, tag="t12")
            nc.tensor.transpose(t12[:, 0:B], ht[:, 0:128], idn)
            nc.tensor.transpose(t12[0:FC_HID - 128, B:2 * B],
                                ht[:, 128:FC_HID], idn)
            h1 = pool.tile([128, B], F32, tag="h1")
            nc.vector.tensor_copy(h1[:], t12[:, 0:B])
            nc.vector.tensor_copy(h2[0:FC_HID - 128, :],
                                  t12[0:FC_HID - 128, B:2 * B])

            # --- fc2 (hidden stationary -> logits [b, cls]) + log_softmax ---
            ps2 = t12[0:B, 2 * B:2 * B + N_CLS]
            nc.tensor.matmul(ps2, h1[:], consts[:, 16:26],
                             start=True, stop=False)
            nc.tensor.matmul(ps2, h2[:], consts[0:96, 26:36],
                             start=False, stop=True)
            ex = pool.tile([B, N_CLS], F32, tag="ex")
            nc.scalar.activation(ex[:], ps2, AF.Exp)
            sm = pool.tile([B, 1], F32, tag="sm")
            nc.vector.reduce_sum(sm[:], ex[:], axis=mybir.AxisListType.X)
            lse = pool.tile([B, 1], F32, tag="lse")
            nc.scalar.activation(lse[:], sm[:], AF.Ln)
            res = pool.tile([B, N_CLS], F32, tag="res")
            nc.vector.tensor_scalar_sub(res[:], ps2, lse[:])
            nc.sync.dma_start(out_d[:], res[:])
    nc.compile()
    return nc


_SLOTS = None
_PROG_CACHE = {}


def kernel(x, src, tgt, conv_w, conv_b, fc1_w, fc1_b, fc2_w, fc2_b):
    global _SLOTS, LAST_RESULT
    x = np.asarray(x, np.float32)
    src = np.asarray(src, np.int32)
    tgt = np.asarray(tgt, np.int32)
    conv_w = np.asarray(conv_w, np.float32)
    conv_b = np.asarray(conv_b, np.float32)
    fc1_w = np.asarray(fc1_w, np.float32)
    fc1_b = np.asarray(fc1_b, np.float32)
    fc2_w = np.asarray(fc2_w, np.float32)
    fc2_b = np.asarray(fc2_b, np.float32)

    steps, fc_live = _cone(src, tgt)
    mainh0, tb, consts, cbt, f1p, slots = _pack(steps, fc_live, conv_w, conv_b,
                                                fc1_w, fc1_b, fc2_w, fc2_b)
    _SLOTS = slots
    ncolsA = slots['xs']

    key = (tuple((v, tuple(ch)) for v, ch in steps), tuple(fc_live), ncolsA)
    if key not in _PROG_CACHE:
        _PROG_CACHE[key] = _build(steps, fc_live, ncolsA, len(fc_live))
    nc = _PROG_CACHE[key]

    xs = x[:, 0]  # [128, 28, 28]
    in_maps = []
    for c in range(N_CORES):
        st = _xstack(xs[c * B:(c + 1) * B])
        mainh = mainh0.copy()
        mainh[:, ncolsA:ncolsA + 512] = st[:128]
        tbc = tb.copy()
        tbc[:, ncolsA:ncolsA + 512] = st[128:160]
        in_maps.append({"mainh": mainh, "tb": tbc, "consts": consts,
                        "cbt": cbt, "f1w": f1p})

    LAST_RESULT = run_bass_kernel_spmd(nc, in_maps, list(range(N_CORES)))
    out = np.concatenate([r["out"] for r in LAST_RESULT.results], axis=0)
    return out.astype(np.float32)


# revision 16
# speedup vs baseline: 1.1787x; 1.0100x over previous
"""Trainium2 Bass kernel for nn_Net_21852793602541 (gnn_message_passing).

The reference net's output depends only on a tiny dependency cone of the
message-passing graph: the final hidden layer reads the wave-2 snapshot of
neuron activations, so only neurons feeding neuron 255 through channels whose
source was already processed matter.  For the fixed graph that is a 3-conv
chain (x -> n0 -> n172 -> n215), one 784->200 FC block, a 200->10 FC and
log_softmax.  The cone is recomputed at runtime from the src/tgt inputs.

Per-core mapping (data-parallel over batch, 16 images/core on 8 cores):
  * 5x5 conv == one PE accumulation group: contraction K = (dy, slot-row)
    with a banded-Toeplitz stationary (fp16) against 5 y-shifted slot copies
    of the padded image block (fp16), N = (batch, y) = 448.  The B-part
    stationary carries a 33rd row holding the conv bias against an all-ones
    row of the moving operand, so PSUM already contains the bias and the
    relu is a plain max(0, x).
  * the relu is split PSUM-side between ACT and DVE (y halves); the slot
    replication copies are spread over DVE (3) and GPSIMD (1); the B-part
    matmul of the next conv is issued first so it can start on the earliest
    copy.
  * fc1 streams the 200 hidden units as the moving operand (7 accumulated
    matmuls); fc1 bias rides a ones-row of the fc stack.  The [16, 200]
    result is relu'd on DVE, transposed via PE, and fc2 uses the hidden
    vectors as the *stationary* so logits land [batch, cls] with no final
    transpose; fc2 bias rides a ones-row of the second hidden chunk.
  * log_softmax without max-subtraction (logits are O(5)): ACT exp, DVE
    reduce, ACT ln, one fused DVE subtract.
  * a Bacc subclass pins the activation-table pass to the
    natural_log_exp_and_others set (covers Relu/Identity/Exp/Ln/Copy), so
    exactly one ACT_TABLE_LOAD runs, early and off the critical path.
"""

import numpy as np

import concourse.bass as bass
import concourse.tile as tile
from concourse import bacc, mybir
from concourse.bacc import _bass_rust
from concourse.bass_utils import run_bass_kernel_spmd
from concourse.hw_specs import get_activation_tables

# The axon NTFF profile hook normally lives in antenv.axon_hooks, which this
# image lacks.  Shim it from the boot module's ctypes implementation so
# BASS_TRACE=1 profiling works; degrade silently if unavailable.
try:
    import antenv.axon_hooks  # noqa: F401
except ImportError:
    try:
        import sys as _sys
        import types as _types

        from trn_agent_boot.trn_boot import _ntff_profile_via_ctypes

        _hook = _ntff_profile_via_ctypes('/opt/axon/libaxon_pjrt.so')
        _mod = _types.ModuleType('antenv.axon_hooks')
        _mod.get_axon_ntff_profile_hook = lambda: _hook
        _mod.set_axon_ntff_profile_hook = lambda h: None
        _sys.modules['antenv.axon_hooks'] = _mod
    except Exception:
        pass

F32 = mybir.dt.float32
F16 = mybir.dt.float16
AF = mybir.ActivationFunctionType
ALU = mybir.AluOpType
N_NEURONS = 256
N_CORES = 8
B_TOTAL = 128
B = B_TOTAL // N_CORES  # 16 images per core
HW = 28
FC_HID = 200
N_CLS = 10

_ACT_SET = 'natural_log_exp_and_others'

LAST_RESULT = None  # BassKernelResults of the most recent run (for profiling)


class _Bacc(bacc.Bacc):
    """Bacc whose activation-table pass only ever picks the combined
    exp+ln set, so the whole kernel needs a single ACT_TABLE_LOAD (the
    greedy first-match pass would otherwise load exp_and_others early and
    natural_log right between the tail's Exp and Ln)."""

    def insert_act_table_loads(self):
        has_activation = any(
            isinstance(i, mybir.InstActivation)
            for b in self.main_func.blocks
            for i in b.instructions
        )
        if not has_activation:
            return
        tables = get_activation_tables(self.m.arch)
        filt = [(name, (fns if name == _ACT_SET else set()))
                for name, fns in tables.items()]
        _bass_rust.insert_act_table_loads(self, filt)


# ---------------------------------------------------------------- schedule
def _schedule(src, tgt):
    n = N_NEURONS
    in_lists = [src[np.where(tgt == i)[0]].astype(np.int64).tolist() for i in range(n)]
    waves = []
    processed = np.zeros(n, bool)
    frontier = [0]
    while True:
        waves.append(list(frontier))
        processed[frontier] = True
        if processed[n - 1]:
            break
        nxt = set()
        for v in frontier:
            for m in tgt[src == v]:
                if not processed[m]:
                    nxt.add(int(m))
        frontier = sorted(nxt)
        assert frontier, "last neuron unreachable"
    return in_lists, waves


def _cone(src, tgt):
    """Returns (steps, fc_live).

    steps: ordered list of (node, [(srckey, channel), ...]) where srckey is
      'x' for the image input or an int neuron id computed in an earlier step.
    fc_live: [(channel_of_255, src_node), ...] live channels of the readout.
    """
    n = N_NEURONS
    in_lists, waves = _schedule(src, tgt)
    wave_of = {}
    for wi, w in enumerate(waves):
        for v in w:
            if v not in wave_of:
                wave_of[v] = wi
    BIG = 1 << 30
    w255 = wave_of[n - 1]
    fc_live = [(c, int(s)) for c, s in enumerate(in_lists[n - 1])
               if wave_of.get(int(s), BIG) < w255]

    live = {}
    stack = [s for _, s in fc_live]
    seen = set()
    while stack:
        v = stack.pop()
        if v in seen:
            continue
        seen.add(v)
        if v == 0:
            live[0] = [('x', 0)]
            continue
        chans = [(int(s), c) for c, s in enumerate(in_lists[v])
                 if wave_of.get(int(s), BIG) < wave_of[v]]
        assert chans, f"cone node {v} has no live channels"
        live[v] = [(s, c) for s, c in chans]
        stack += [s for s, _ in chans]

    steps = sorted(live.items(), key=lambda kv: wave_of[kv[0]])
    return steps, fc_live


# ---------------------------------------------------------- host-side packing
def _toeplitz(w):
    """w [5,5] -> [160, 28] banded matrix over K=(dy, row).

    Slot row r of each 32-row group holds padded-image column (r+2) mod 32,
    so the activation value at x lands at row x (32-aligned writes; wrapped
    rows 28..31 hold the zero x-padding)."""
    T = np.zeros((160, HW), np.float32)
    for dy in range(5):
        for dx in range(5):
            for xc in range(HW):
                T[dy * 32 + (xc + dx - 2) % 32, xc] = w[dy, dx]
    return T


def _xstack(xb):
    """xb [B,28,28] -> [160, B*32] fp16: five y-shifted padded slot copies.

    Slot_dy[r, b*32+yp] = xpad[b, yp+dy-2, (r+2) % 32]."""
    xpad = np.zeros((B, 32, 32), np.float32)
    xpad[:, 2:30, 2:30] = xb
    st = np.zeros((5, 32, B, 32), np.float32)
    for dy in range(5):
        lo, hi = max(0, 2 - dy), min(32, 34 - dy)
        st[dy, :, :, lo:hi] = xpad[:, lo + dy - 2:hi + dy - 2, :].transpose(2, 0, 1)
    st = np.roll(st, -2, axis=1)
    return st.reshape(160, B * 32).astype(np.float16)


def _pack(steps, fc_live, conv_w, conv_b, fc1_w, fc1_b, fc2_w, fc2_b):
    """Builds mainh (f16, [128, *]), tb (f16, [33, *]), consts (f32,
    [128, 36]), f1w (f16, [128, 1400*nfc]) and the slot map."""
    slots = {}
    col = 0
    for v, chans in steps:
        for j, _ in enumerate(chans):
            slots[('toep', v, j)] = col
            col += HW
    a_cols = col
    slots['xs'] = a_cols
    n_toep = a_cols  # same column budget in tb (B parts align with A parts)

    mainh = np.zeros((128, a_cols + 512), np.float16)
    tb = np.zeros((32, a_cols + 512), np.float16)
    for v, chans in steps:
        for j, (skey, ch) in enumerate(chans):
            T = _toeplitz(conv_w[v, 0, ch])
            c0 = slots[('toep', v, j)]
            mainh[:, c0:c0 + HW] = T[:128]
            tb[:, c0:c0 + HW] = T[128:]

    # consts: identity (16x16), fc2wA [128,10], fc2wB [96,10] w/ bias row;
    # conv biases ride a separate tiny tensor so they land early
    nsteps = len(steps)
    consts = np.zeros((128, 36), np.float32)
    consts[:B, 0:B] = np.eye(B, dtype=np.float32)
    w2t = fc2_w.T  # [200, 10]
    consts[:, 16:26] = w2t[:128]
    consts[:FC_HID - 128, 26:36] = w2t[128:]
    consts[72, 26:36] = fc2_b
    # padded to 32 rows: descriptor generation has a fast path only for
    # multiple-of-16 partition counts
    cbt = np.zeros((32, nsteps), np.float32)
    for i, (v, _) in enumerate(steps):
        slots[('cb', v)] = i
        cbt[:HW, i] = conv_b[v]

    n_fc = len(fc_live)
    f1p = np.zeros((128, 1400 * n_fc), np.float16)
    for k, (c, s) in enumerate(fc_live):
        blk = fc1_w[:, c * 784:(c + 1) * 784].reshape(FC_HID, HW, HW)  # [h, y, x]
        arr = blk.reshape(FC_HID, 4, 7, HW).transpose(1, 3, 2, 0)  # [yg, x, ysub, h]
        f1p[:, k * 1400:(k + 1) * 1400] = np.pad(
            arr, ((0, 0), (0, 4), (0, 0), (0, 0))).reshape(128, 1400)
    f1p[28, 0:FC_HID] = fc1_b  # vs ones-row of fc stack (k=0, sj=0 only)
    return mainh, tb, consts, cbt, f1p, slots


# ---------------------------------------------------------- device program
def _build(steps, fc_live, ncolsA, nfc):
    nc = _Bacc("TRN2", target_bir_lowering=False)
    nsteps = len(steps)
    mainh_d = nc.dram_tensor("mainh", [128, ncolsA + 512], F16, kind="ExternalInput")
    tb_d = nc.dram_tensor("tb", [32, ncolsA + 512], F16, kind="ExternalInput")
    consts_d = nc.dram_tensor("consts", [128, 36], F32, kind="ExternalInput")
    cbt_d = nc.dram_tensor("cbt", [32, nsteps], F32, kind="ExternalInput")
    f1w_d = nc.dram_tensor("f1w", [128, 1400 * nfc], F16, kind="ExternalInput")
    out_d = nc.dram_tensor("out", [B, N_CLS], F32, kind="ExternalOutput")

    feeds_conv = set()
    for v, chans in steps:
        for skey, _ in chans:
            if skey != 'x':
                feeds_conv.add(skey)
    fc_srcs = [s for _, s in fc_live]
    SL = _SLOTS
    YS = 14  # relu y-split point: ACT gets [0, YS), DVE [YS, 28)

    with tile.TileContext(nc) as tc:
        with (
            tc.tile_pool(name="persist", bufs=1) as pool,
            tc.tile_pool(name="cpsum", bufs=1, space="PSUM") as cpp,
            tc.tile_pool(name="qpsum", bufs=1, space="PSUM") as qpp,
            tc.tile_pool(name="fpsum", bufs=1, space="PSUM") as fpp,
        ):
            mainh = pool.tile([128, ncolsA + 512], F16, tag="mainh")
            tbt = pool.tile([32, ncolsA + 512], F16, tag="tb")
            consts = pool.tile([128, 36], F32, tag="consts")
            cbt = pool.tile([32, nsteps], F32, tag="cbt")
            f1w = pool.tile([128, 1400 * nfc], F16, tag="f1w")
            # mainh gates conv0; f1w is not needed until fc1 -- both on the
            # HWDGE sync queue (mainh first), small tensors on scalar.
            nc.sync.dma_start(mainh[:], mainh_d[:])
            nc.sync.dma_start(f1w[:], f1w_d[:])
            nc.scalar.dma_start(tbt[:], tb_d[:])
            nc.scalar.dma_start(consts[:], consts_d[:])
            nc.gpsimd.dma_start(cbt[:], cbt_d[:])

            # activation slot tiles per conv producer (fp16, zero borders);
            # memsets split DVE/GPSIMD so neither engine's prologue is long
            stacks = {}
            ms_engines = [nc.vector, nc.gpsimd]
            for mi, v in enumerate(sorted(feeds_conv)):
                eng = ms_engines[mi % 2]
                a = pool.tile([128, B * 32], F16, name=f"stA_{v}", tag=f"stA_{v}")
                b = pool.tile([32, B * 32], F16, name=f"stB_{v}", tag=f"stB_{v}")
                eng.memset(a[:], 0.0)
                eng.memset(b[:], 0.0)
                stacks[v] = (a, b)
            fcstacks = {}
            for sv in sorted(set(fc_srcs)):
                t = pool.tile([128, B * 7], F16, name=f"fcst_{sv}", tag=f"fcst_{sv}")
                nc.gpsimd.memset(t[:], 0.0)
                fv = t[:].rearrange("p (b s) -> p b s", s=7)
                # ones-row for the fc1 bias trick; partition starts must be
                # 32-aligned, so write rows 0:32 -- rows 0:28 are overwritten
                # by the g=0 quarter write, rows 29:31 face zero-padded
                # stationary rows
                nc.gpsimd.memset(fv[0:32, :, 0:1], 1.0)
                fcstacks[sv] = t
            h2 = pool.tile([96, B], F32, tag="h2")
            nc.gpsimd.memset(h2[64:96, :], 1.0)

            xsa = mainh[:, SL['xs']:SL['xs'] + 512]
            xsb = tbt[:, SL['xs']:SL['xs'] + 512]

            def slot_slices(key):
                if key == 'x':
                    av, bv = xsa, xsb
                else:
                    a, b = stacks[key]
                    av, bv = a[:], b[:]
                return (av.rearrange("p (b y) -> p b y", y=32),
                        bv.rearrange("p (b y) -> p b y", y=32))

            # --- conv chain ---
            for v, chans in steps:
                nch = len(chans)
                cb = cbt[:HW, SL[('cb', v)]:SL[('cb', v)] + 1]
                fc_only = v in fcstacks and v not in feeds_conv

                if fc_only:
                    # four independent quarter PSUMs: each regroup write has
                    # its own tile, so the DVE/ACT pairs truly overlap
                    fst = fcstacks[v]
                    fv = fst[:].rearrange("p (b s) -> p b s", s=7)
                    # four PSUM tiles in four banks: concurrently-open
                    # accumulation groups must not share a PSUM bank, and
                    # separate tiles keep the regroup writes parallel
                    qs = [qpp.tile([HW, B * 7], F32, tag=f"q{g}",
                                   name=f"q{v}_{g}")[:]
                          for g in range(4)]
                    for j, (skey, ch) in enumerate(chans):
                        c0 = SL[('toep', v, j)]
                        av, bv = slot_slices(skey)
                        for g in range(4):
                            ysl = slice(2 + 7 * g, 9 + 7 * g)
                            nc.tensor.matmul(qs[g], tbt[:, c0:c0 + HW],
                                             bv[:, :, ysl],
                                             start=(j == 0), stop=False)
                        for g in range(4):
                            ysl = slice(2 + 7 * g, 9 + 7 * g)
                            nc.tensor.matmul(qs[g], mainh[:, c0:c0 + HW],
                                             av[:, :, ysl],
                                             start=False, stop=(j == nch - 1))
                    for g in range(4):
                        dst = fv[g * 32:g * 32 + HW, :, :]
                        qv = qs[g].rearrange("p (b y) -> p b y", y=7)
                        if g % 2 == 0:
                            nc.vector.tensor_scalar(dst, qv, cb, 0.0,
                                                    ALU.add, ALU.max)
                        else:
                            nc.scalar.activation(dst, qv, AF.Relu, bias=cb,
                                                 scale=1.0)
                    continue

                # y-split PSUM halves: ACT relus the low half while DVE does
                # the high half -- separate tiles, so no reader serialization
                plo = cpp.tile([HW, B * YS], F32, tag="pslo", name=f"plo{v}")
                phi = cpp.tile([HW, B * (HW - YS)], F32, tag="pshi",
                               name=f"phi{v}")
                for j, (skey, ch) in enumerate(chans):
                    c0 = SL[('toep', v, j)]
                    av, bv = slot_slices(skey)
                    # B part first: its moving operand is copied first, so
                    # the PE can start before the dy slot copies finish
                    nc.tensor.matmul(plo[:], tbt[:, c0:c0 + HW],
                                     bv[:, :, 2:2 + YS],
                                     start=(j == 0), stop=False)
                    nc.tensor.matmul(phi[:], tbt[:, c0:c0 + HW],
                                     bv[:, :, 2 + YS:30],
                                     start=(j == 0), stop=False)
                    nc.tensor.matmul(plo[:], mainh[:, c0:c0 + HW],
                                     av[:, :, 2:2 + YS],
                                     start=False, stop=(j == nch - 1))
                    nc.tensor.matmul(phi[:], mainh[:, c0:c0 + HW],
                                     av[:, :, 2 + YS:30],
                                     start=False, stop=(j == nch - 1))
                plov = plo[:].rearrange("p (b y) -> p b y", y=YS)
                phiv = phi[:].rearrange("p (b y) -> p b y", y=HW - YS)

                av, bv = slot_slices(v)
                g2 = av[64:64 + HW, :, 2:30]
                nc.scalar.activation(av[64:64 + HW, :, 2:2 + YS], plov,
                                     AF.Relu, bias=cb, scale=1.0)
                nc.vector.tensor_scalar(av[64:64 + HW, :, 2 + YS:30], phiv,
                                        cb, 0.0, ALU.add, ALU.max)
                nc.vector.tensor_copy(bv[0:HW, :, 0:28], g2)
                nc.vector.tensor_copy(av[0:HW, :, 4:32], g2)
                nc.vector.tensor_copy(av[32:32 + HW, :, 3:31], g2)
                nc.scalar.copy(av[96:96 + HW, :, 1:29], g2)
                if v in fcstacks:  # node feeds both conv and fc (rare)
                    fst = fcstacks[v]
                    fv = fst[:].rearrange("p (b s) -> p b s", s=7)
                    for g in range(4):
                        dst = fv[g * 32:g * 32 + HW, :, :]
                        lo, hi = 7 * g, 7 * g + 7
                        if hi <= YS:
                            src3 = plov[:, :, lo:hi]
                        elif lo >= YS:
                            src3 = phiv[:, :, lo - YS:hi - YS]
                        else:
                            nc.vector.tensor_scalar(dst[:, :, 0:YS - lo],
                                                    plov[:, :, lo:YS], cb,
                                                    0.0, ALU.add, ALU.max)
                            nc.vector.tensor_scalar(dst[:, :, YS - lo:],
                                                    phiv[:, :, 0:hi - YS], cb,
                                                    0.0, ALU.add, ALU.max)
                            continue
                        nc.vector.tensor_scalar(dst, src3, cb, 0.0,
                                                ALU.add, ALU.max)

            # --- fc1: activations stationary, hidden units streamed ---
            p1 = fpp.tile([B, FC_HID], F32, tag="p1")
            for k in range(nfc):
                fst = fcstacks[fc_live[k][1]]
                fv = fst[:].rearrange("p (b s) -> p b s", s=7)
                for sj in range(7):
                    i = k * 7 + sj
                    nc.tensor.matmul(p1[:], fv[:, :, sj:sj + 1],
                                     f1w[:, (k * 7 + sj) * 200:(k * 7 + sj + 1) * 200],
                                     start=(i == 0), stop=(i == 7 * nfc - 1))
            # bias already accumulated; relu+copy in one DVE op
            ht = pool.tile([B, FC_HID], F32, tag="ht")
            nc.vector.tensor_scalar_max(ht[:], p1[:], 0.0)
            idn = consts[:B, 0:B]
            # t1/t2/ps2 share one PSUM bank: their matmul groups never
            # overlap in time (transposes close before fc2 starts)
            t12 = fpp.tile([128, 2 * B + N_CLS], F32, tag="t12")
            nc.tensor.transpose(t12[:, 0:B], ht[:, 0:128], idn)
            nc.tensor.transpose(t12[0:FC_HID - 128, B:2 * B],
                                ht[:, 128:FC_HID], idn)
            h1 = pool.tile([128, B], F32, tag="h1")
            nc.vector.tensor_copy(h1[:], t12[:, 0:B])
            nc.vector.tensor_copy(h2[0:FC_HID - 128, :],
                                  t12[0:FC_HID - 128, B:2 * B])

            # --- fc2 (hidden stationary -> logits [b, cls]) + log_softmax ---
            ps2 = t12[0:B, 2 * B:2 * B + N_CLS]
            nc.tensor.matmul(ps2, h1[:], consts[:, 16:26],
                             start=True, stop=False)
            nc.tensor.matmul(ps2, h2[:], consts[0:96, 26:36],
                             start=False, stop=True)
            ex = pool.tile([B, N_CLS], F32, tag="ex")
            sm = pool.tile([B, 1], F32, tag="sm")
            nc.scalar.activation(ex[:], ps2, AF.Exp, accum_out=sm[:])
            lse = pool.tile([B, 1], F32, tag="lse")
            nc.scalar.activation(lse[:], sm[:], AF.Ln)
            res = pool.tile([B, N_CLS], F32, tag="res")
            nc.vector.tensor_scalar_sub(res[:], ps2, lse[:])
            nc.sync.dma_start(out_d[:], res[:])
    nc.compile()
    return nc


_SLOTS = None
_PROG_CACHE = {}


def kernel(x, src, tgt, conv_w, conv_b, fc1_w, fc1_b, fc2_w, fc2_b):
    global _SLOTS, LAST_RESULT
    x = np.asarray(x, np.float32)
    src = np.asarray(src, np.int32)
    tgt = np.asarray(tgt, np.int32)
    conv_w = np.asarray(conv_w, np.float32)
    conv_b = np.asarray(conv_b, np.float32)
    fc1_w = np.asarray(fc1_w, np.float32)
    fc1_b = np.asarray(fc1_b, np.float32)
    fc2_w = np.asarray(fc2_w, np.float32)
    fc2_b = np.asarray(fc2_b, np.float32)

    steps, fc_live = _cone(src, tgt)
    mainh0, tb, consts, cbt, f1p, slots = _pack(steps, fc_live, conv_w, conv_b,
                                                fc1_w, fc1_b, fc2_w, fc2_b)
    _SLOTS = slots
    ncolsA = slots['xs']

    key = (tuple((v, tuple(ch)) for v, ch in steps), tuple(fc_live), ncolsA)
    if key not in _PROG_CACHE:
        _PROG_CACHE[key] = _build(steps, fc_live, ncolsA, len(fc_live))
    nc = _PROG_CACHE[key]

    xs = x[:, 0]  # [128, 28, 28]
    in_maps = []
    for c in range(N_CORES):
        st = _xstack(xs[c * B:(c + 1) * B])
        mainh = mainh0.copy()
        mainh[:, ncolsA:ncolsA + 512] = st[:128]
        tbc = tb.copy()
        tbc[:, ncolsA:ncolsA + 512] = st[128:160]
        in_maps.append({"mainh": mainh, "tb": tbc, "consts": consts,
                        "cbt": cbt, "f1w": f1p})

    LAST_RESULT = run_bass_kernel_spmd(nc, in_maps, list(range(N_CORES)))
    out = np.concatenate([r["out"] for r in LAST_RESULT.results], axis=0)
    return out.astype(np.float32)
